# revision 25
# baseline (speedup 1.0000x reference)
"""Trainium2 Bass kernel for Swin-style window attention with relative position bias.

Problem (hardcoded): x[64,256,256] f32, w_qkv[256,768], bias_table[961,8],
w_out[256,256], b_out[256], rel_index[65536] int32.
out = proj(softmax(q k^T/sqrt(d) + bias) v) with 8 heads of dim 32.

Strategy: data-parallel over batch across 8 cores (8 batches per core).
All on-chip per batch, transposed-attention layout:
  - load x^T via strided DMA, project to qT/kT (head-dim on partitions) and
    v natural; dots computed transposed [j, n] so softmax sums come free from
    a ones-column folded into the attn@v matmul (M=33 per head).
  - exp(dots) on ACT, multiplied by host-precomputed exp(bias^T) on GpSimd.
  - normalization: per-head sums rows gathered by DMA, reciprocal on DVE,
    broadcast via DMA to D tiles, applied in one fused TT per pair-tile.
  - matmuls in float32r (tf32-class, 1 cycle/row at N>=256).
"""
import os
import sys

for _p in ("/opt/trn_rl_repo", "/root/.axon_site/_ro/trn_rl_repo"):
    if os.path.isdir(_p) and _p not in sys.path:
        sys.path.insert(0, _p)

import numpy as np

import concourse.bass as bass
import concourse.mybir as mybir
import concourse.tile as tile
from concourse import bacc
from concourse.bass_utils import run_bass_kernel_spmd
from concourse.masks import make_identity

N_CORES = 8
B = 64
BPC = B // N_CORES          # batches per core
N = 256                     # tokens
H = 8                       # heads
D = 32                      # head dim
C = 256                     # channels (in = inner = out)
SCALE = D ** -0.5

F32 = mybir.dt.float32
F32R = mybir.dt.float32r
BF16 = mybir.dt.bfloat16


def build_nc(reps: int = 1):
    nc = bacc.Bacc("TRN2", target_bir_lowering=False, debug=False,
                   num_devices=N_CORES)

    xt8 = nc.dram_tensor("xt8", [BPC, C, N], F32, kind="ExternalInput").ap()
    wq = nc.dram_tensor("wq", [128, 2, C], F32, kind="ExternalInput").ap()
    wk = nc.dram_tensor("wk", [128, 2, C], F32, kind="ExternalInput").ap()
    wv = nc.dram_tensor("wv", [128, 2, C], F32, kind="ExternalInput").ap()
    wo = nc.dram_tensor("wo", [128, 2, C], F32, kind="ExternalInput").ap()
    bo = nc.dram_tensor("bo", [C], F32, kind="ExternalInput").ap()
    ebt = nc.dram_tensor("ebt", [128, H, 2, N], F32, kind="ExternalInput").ap()
    out8 = nc.dram_tensor("out8", [BPC, N, C], F32, kind="ExternalOutput").ap()

    with tile.TileContext(nc) as tc:
        with (
            tc.tile_pool(name="singles", bufs=1) as singles,
            tc.tile_pool(name="work", bufs=2) as work,
            tc.tile_pool(name="ework", bufs=3) as ework,
            tc.tile_pool(name="stagp", bufs=4) as stagp,
            tc.tile_pool(name="pp", bufs=1, space="PSUM") as pp,
            tc.tile_pool(name="vf", bufs=2, space="PSUM") as vfp,
            tc.tile_pool(name="dp", bufs=2, space="PSUM") as dp,
            tc.tile_pool(name="natp", bufs=1, space="PSUM") as natp,
        ):
            # ---- one-time prologue: weights, tables, constants ----
            wq_r = singles.tile([128, 2, C], F32R, tag="wq")
            wk_r = singles.tile([128, 2, C], F32R, tag="wk")
            wv_r = singles.tile([128, 2, C], F32R, tag="wv")
            wo_r = singles.tile([128, 2, C], F32R, tag="wo")
            bb = singles.tile([128, C], F32, tag="bb")
            ebt_sb = singles.tile([128, H, 2, N], F32, tag="ebt")

            for wi, (dst, wsrc) in enumerate(
                    ((wq_r, wq), (wk_r, wk), (wv_r, wv), (wo_r, wo))):
                tmp = singles.tile([128, 4, C], F32, tag=f"wtmp{wi}",
                                   name=f"wtmp{wi}")
                fs = wsrc.shape[1]  # 2 or 4
                nc.sync.dma_start(out=tmp[:, :fs, :], in_=wsrc[:])
                nc.vector.tensor_copy(dst[:], tmp[:, :fs, :])
            bo_bcast = bass.AP(tensor=bo.tensor, offset=bo.offset,
                               ap=[[0, 128]] + list(bo.ap))
            nc.sync.dma_start(out=bb[:], in_=bo_bcast)
            for hh in range(2):
                nc.sync.dma_start(out=ebt_sb[:, 4 * hh:4 * hh + 4, :, :],
                                  in_=ebt[:, 4 * hh:4 * hh + 4, :, :])

            # persistent double-buffered tensors
            v_augs = [singles.tile([128, 2, H * 34], F32R, tag=f"vaug{i}",
                                   name=f"vaug{i}") for i in range(2)]
            ones_f = singles.tile([128, 2, H, 2], F32, tag="onesf")
            nc.vector.memset(ones_f[:, :, :, 0:1], 1.0)
            nc.vector.memset(ones_f[:, :, :, 1:2], 0.0)
            for va in v_augs:
                pad_ap = va.rearrange("p jc (h w) -> p jc h w", w=34)[:, :, :, 32:34]
                nc.vector.tensor_copy(pad_ap, ones_f[:])
            ident_f = singles.tile([128, 128], F32, tag="identf")
            make_identity(nc, ident_f[:])
            ident_r = singles.tile([128, 128], F32R, tag="identr")
            nc.vector.tensor_copy(ident_r[:], ident_f[:])

            def load_xt(b):
                xt_f = work.tile([128, 2, N], F32, tag="xtf", name="xtf",
                                 bufs=BPC)
                nc.sync.dma_start(
                    out=xt_f[:], in_=xt8[b].rearrange("(cc p) n -> p cc n", p=128))
                xt = work.tile([128, 2, N], F32R, tag="xtr", name="xtr",
                               bufs=BPC)
                nc.vector.tensor_copy(xt[:], xt_f[:])
                return xt

            def batch_body(b, xt):
                va = v_augs[b % 2]

                # ---- projections ----
                # qkT: [c_out-chunk mc (128p), n] = sum_cc w[:,cc,mc].T @ xT[cc]
                # qk psum layout [128, (q mc0, q mc1, k mc0, k mc1), n]
                qkt = work.tile([128, 4, N], F32R, tag="qkt")
                ps_qk = pp.tile([128, 4, N], F32, tag="qk")
                for wi, w_r in enumerate((wq_r, wk_r)):
                    for mc in range(2):
                        for cc in range(2):
                            nc.tensor.matmul(
                                ps_qk[:, 2 * wi + mc, :],
                                w_r[:, cc, mc * 128:(mc + 1) * 128],
                                xt[:, cc, :],
                                start=(cc == 0), stop=(cc == 1),
                            )
                nc.vector.tensor_copy(qkt[:], ps_qk[:])
                # v natural: [j-chunk jc (128p), d_all] = sum_cc xT[cc,:,jslice].T @ wv[cc]
                ps_v = vfp.tile([128, 2, N], F32, tag="vf")
                for jc in range(2):
                    for cc in range(2):
                        nc.tensor.matmul(
                            ps_v[:, jc, :],
                            xt[:, cc, jc * 128:(jc + 1) * 128],
                            wv_r[:, cc, :],
                            start=(cc == 0), stop=(cc == 1),
                        )
                nc.vector.tensor_copy(
                    va.rearrange("p jc (h w) -> p jc h w", w=34)[:, :, :, 0:32],
                    ps_v.rearrange("p jc (h d) -> p jc h d", d=32),
                )

                # ---- attention, per head; attn@v lands in natural layout ----
                nat = natp.tile([128, 2, H, 64], F32, tag="nat", name="nat")
                for h in range(H):
                    rb = (h % 4) * 32
                    mc = h // 4
                    dots = dp.tile([128, 2, N], F32, tag="dots", name="dots")
                    for jc in range(2):
                        nc.tensor.matmul(
                            dots[:, jc, :],
                            qkt[rb:rb + 32, 2 + mc, jc * 128:(jc + 1) * 128],
                            qkt[rb:rb + 32, mc, :],
                            start=True, stop=True,
                            tile_position=(rb, 0),
                        )
                    eraw = ework.tile([128, 2, N], F32, tag="eraw", name="eraw")
                    nc.scalar.activation(eraw[:], dots[:],
                                         mybir.ActivationFunctionType.Exp)
                    e_h = ework.tile([128, 2, N], F32R, tag="e", name="e")
                    nc.gpsimd.tensor_mul(e_h[:], eraw[:], ebt_sb[:, h, :, :])
                    # out_nat[n, d] (+ sums in column 32) = E^T @ [v | 1]
                    for nck in range(2):
                        for jc in range(2):
                            nc.tensor.matmul(
                                nat[:, nck, h, 0:34],
                                e_h[:, jc, nck * 128:(nck + 1) * 128],
                                va[:, jc, h * 34:(h + 1) * 34],
                                start=(jc == 0), stop=(jc == 1),
                            )

                # ---- softmax normalization: per-partition scalars ----
                r_s = work.tile([128, 2, H, 1], F32, tag="r", name="r")
                nc.vector.reciprocal_approx_fast(
                    r_s.rearrange("p nck h o -> p (nck h o)"),
                    nat[:, :, :, 32:33].rearrange("p nck h o -> p (nck h o)"))
                stag = stagp.tile([128, 2, H, 32], F32R, tag="stag", name="stag")
                nc.vector.tensor_mul(
                    stag[:], nat[:, :, :, 0:32],
                    r_s.broadcast_to([128, 2, H, 32]))

                # ---- transpose to [inner, n] for the output projection ----
                sv = stag.rearrange("p nck h d -> p nck (h d)")
                psT = dp.tile([128, 2, 2, 128], F32R, tag="dots", name="psT")
                for nck in range(2):
                    for ic in range(2):
                        nc.tensor.transpose(
                            psT[:, ic, nck, :],
                            sv[:, nck, ic * 128:(ic + 1) * 128],
                            ident_r[:])
                stag_t = stagp.tile([128, 2, 2, 128], F32R, tag="stagT",
                                    name="stagT")
                nc.vector.tensor_copy(stag_t[:], psT[:])

                # ---- output projection ----
                ps_f = vfp.tile([128, 2, N], F32, tag="vf")
                for nck in range(2):
                    for ic in range(2):
                        nc.tensor.matmul(
                            ps_f[:, nck, :],
                            stag_t[:, ic, nck, :],
                            wo_r[:, ic, :],
                            start=(ic == 0), stop=(ic == 1),
                        )
                fout = work.tile([128, 2, N], F32, tag="fout")
                nc.vector.tensor_add(
                    fout[:], ps_f[:],
                    bb[:, None, :].broadcast_to([128, 2, N]),
                )
                nc.sync.dma_start(
                    out=out8[b].rearrange("(ncc p) c -> p ncc c", p=128),
                    in_=fout[:],
                )

            if reps == 1:
                xts = [load_xt(b) for b in range(BPC)]
                for b in range(BPC):
                    batch_body(b, xts[b])
            else:
                with tc.For_i(0, reps, 1,
                              hint_engines=(mybir.EngineType.PE,)):
                    xts = [load_xt(b) for b in range(BPC)]
                    for b in range(BPC):
                        batch_body(b, xts[b])

    nc.compile()
    return nc


def _prep_shared(w_qkv, bias_table, w_out, b_out, rel_index):
    w_qkv = np.asarray(w_qkv, dtype=np.float32)
    w_q = w_qkv[:, 0:C] * np.float32(SCALE)
    w_k = w_qkv[:, C:2 * C]
    w_v = w_qkv[:, 2 * C:3 * C]
    wq_t = np.ascontiguousarray(w_q.reshape(2, 128, C).transpose(1, 0, 2))
    wk_t = np.ascontiguousarray(w_k.reshape(2, 128, C).transpose(1, 0, 2))
    wv_t = np.ascontiguousarray(w_v.reshape(2, 128, C).transpose(1, 0, 2))

    w_out = np.asarray(w_out, dtype=np.float32)
    wo_t = np.ascontiguousarray(w_out.reshape(2, 128, C).transpose(1, 0, 2))

    # exp of relative-position bias, transposed per head: expBT[h, j, n]
    tbl = np.asarray(bias_table, dtype=np.float32)[np.asarray(rel_index)]
    bias_njh = tbl.reshape(N, N, H)                 # [n, j, h]
    ebt_full = np.exp(bias_njh.transpose(2, 1, 0))  # [h, j, n]
    ebt_t = np.ascontiguousarray(
        ebt_full.reshape(H, 2, 128, N).transpose(2, 0, 1, 3))  # [p, h, jc, n]

    return {
        "wq": wq_t, "wk": wk_t, "wv": wv_t, "wo": wo_t,
        "bo": np.ascontiguousarray(np.asarray(b_out, dtype=np.float32)),
        "ebt": ebt_t,
    }


_NC_CACHE = {}


def get_nc(reps: int = 1):
    if reps not in _NC_CACHE:
        _NC_CACHE[reps] = build_nc(reps)
    return _NC_CACHE[reps]


def make_in_maps(x, w_qkv, bias_table, w_out, b_out, rel_index):
    shared = _prep_shared(w_qkv, bias_table, w_out, b_out, rel_index)
    x = np.asarray(x, dtype=np.float32)
    xs = x.reshape(N_CORES, BPC, N, C).transpose(0, 1, 3, 2)
    xs = np.ascontiguousarray(xs)
    return [{"xt8": xs[i], **shared} for i in range(N_CORES)]


def kernel(x, w_qkv, bias_table, w_out, b_out, rel_index):
    nc = get_nc(1)
    in_maps = make_in_maps(x, w_qkv, bias_table, w_out, b_out, rel_index)
    res = run_bass_kernel_spmd(nc, in_maps, core_ids=list(range(N_CORES)))
    return np.concatenate([res.results[i]["out8"] for i in range(N_CORES)],
                          axis=0)


# revision 29
# speedup vs baseline: 99.9925x; 99.9925x over previous
"""Trainium2 Bass kernel for Swin-style window attention with relative position bias.

Problem (hardcoded): x[64,256,256] f32, w_qkv[256,768], bias_table[961,8],
w_out[256,256], b_out[256], rel_index[65536] int32.
out = proj(softmax(q k^T/sqrt(d) + bias) v) with 8 heads of dim 32.

Strategy: data-parallel over batch across 8 cores (8 batches per core).
All on-chip per batch, transposed-attention layout:
  - load x^T via strided DMA, project to qT/kT (head-dim on partitions) and
    v natural; dots computed transposed [j, n] so softmax sums come free from
    a ones-column folded into the attn@v matmul (M=33 per head).
  - exp(dots) on ACT, multiplied by host-precomputed exp(bias^T) on GpSimd.
  - normalization: per-head sums rows gathered by DMA, reciprocal on DVE,
    broadcast via DMA to D tiles, applied in one fused TT per pair-tile.
  - matmuls in float32r (tf32-class, 1 cycle/row at N>=256).
"""
import os
import sys

for _p in ("/opt/trn_rl_repo", "/root/.axon_site/_ro/trn_rl_repo"):
    if os.path.isdir(_p) and _p not in sys.path:
        sys.path.insert(0, _p)

import numpy as np

import concourse.bass as bass
import concourse.mybir as mybir
import concourse.tile as tile
from concourse import bacc
from concourse.bass_utils import run_bass_kernel_spmd
from concourse.masks import make_identity

N_CORES = 8
B = 64
BPC = B // N_CORES          # batches per core
N = 256                     # tokens
H = 8                       # heads
D = 32                      # head dim
C = 256                     # channels (in = inner = out)
SCALE = D ** -0.5

F32 = mybir.dt.float32
F32R = mybir.dt.float32r
BF16 = mybir.dt.bfloat16


def build_nc(reps: int = 1):
    nc = bacc.Bacc("TRN2", target_bir_lowering=False, debug=False,
                   num_devices=N_CORES)

    xt8 = nc.dram_tensor("xt8", [BPC, C, N], F32, kind="ExternalInput").ap()
    wq = nc.dram_tensor("wq", [128, 2, C], F32, kind="ExternalInput").ap()
    wk = nc.dram_tensor("wk", [128, 2, C], F32, kind="ExternalInput").ap()
    wv = nc.dram_tensor("wv", [128, 2, C], F32, kind="ExternalInput").ap()
    wo = nc.dram_tensor("wo", [128, 2, C], F32, kind="ExternalInput").ap()
    bo = nc.dram_tensor("bo", [C], F32, kind="ExternalInput").ap()
    ebt = nc.dram_tensor("ebt", [128, H, 2, N], BF16, kind="ExternalInput").ap()
    out8 = nc.dram_tensor("out8", [BPC, N, C], F32, kind="ExternalOutput").ap()

    with tile.TileContext(nc) as tc:
        with (
            tc.tile_pool(name="singles", bufs=1) as singles,
            tc.tile_pool(name="work", bufs=2) as work,
            tc.tile_pool(name="ework", bufs=4) as ework,
            tc.tile_pool(name="stagp", bufs=4) as stagp,
            tc.tile_pool(name="pp", bufs=1, space="PSUM") as pp,
            tc.tile_pool(name="vf", bufs=1, space="PSUM") as vfp,
            tc.tile_pool(name="dp", bufs=2, space="PSUM") as dp,
            tc.tile_pool(name="natp", bufs=1, space="PSUM") as natp,
        ):
            # ---- one-time prologue: weights, tables, constants ----
            wq_r = singles.tile([128, 2, C], F32R, tag="wq")
            wk_r = singles.tile([128, 2, C], F32R, tag="wk")
            wv_r = singles.tile([128, 2, C], F32R, tag="wv")
            wo_r = singles.tile([128, 2, C], F32R, tag="wo")
            bb = singles.tile([128, C], F32, tag="bb")
            ebt_sb = singles.tile([128, H, 2, N], BF16, tag="ebt")

            for wi, (dst, wsrc) in enumerate(
                    ((wq_r, wq), (wk_r, wk), (wv_r, wv), (wo_r, wo))):
                tmp = singles.tile([128, 4, C], F32, tag=f"wtmp{wi}",
                                   name=f"wtmp{wi}")
                fs = wsrc.shape[1]  # 2 or 4
                nc.sync.dma_start(out=tmp[:, :fs, :], in_=wsrc[:])
                nc.vector.tensor_copy(dst[:], tmp[:, :fs, :])
            bo_bcast = bass.AP(tensor=bo.tensor, offset=bo.offset,
                               ap=[[0, 128]] + list(bo.ap))
            nc.sync.dma_start(out=bb[:], in_=bo_bcast)
            for hh in range(2):
                nc.sync.dma_start(out=ebt_sb[:, 4 * hh:4 * hh + 4, :, :],
                                  in_=ebt[:, 4 * hh:4 * hh + 4, :, :])

            # persistent double-buffered tensors
            v_augs = [singles.tile([128, 2, H * 34], BF16, tag=f"vaug{i}",
                                   name=f"vaug{i}") for i in range(2)]
            ones_f = singles.tile([128, 2, H, 2], F32, tag="onesf")
            nc.vector.memset(ones_f[:, :, :, 0:1], 1.0)
            nc.vector.memset(ones_f[:, :, :, 1:2], 0.0)
            for va in v_augs:
                pad_ap = va.rearrange("p jc (h w) -> p jc h w", w=34)[:, :, :, 32:34]
                nc.vector.tensor_copy(pad_ap, ones_f[:])
            ident_f = singles.tile([128, 128], F32, tag="identf")
            make_identity(nc, ident_f[:])
            ident_r = singles.tile([128, 128], F32R, tag="identr")
            nc.vector.tensor_copy(ident_r[:], ident_f[:])

            def load_xt(b):
                xt_f = work.tile([128, 2, N], F32, tag="xtf", name="xtf",
                                 bufs=BPC)
                nc.sync.dma_start(
                    out=xt_f[:], in_=xt8[b].rearrange("(cc p) n -> p cc n", p=128))
                xt = work.tile([128, 2, N], F32R, tag="xtr", name="xtr",
                               bufs=BPC)
                nc.gpsimd.tensor_copy(xt[:], xt_f[:])
                return xt

            def batch_body(b, xt):
                va = v_augs[b % 2]

                # ---- projections ----
                # qT/kT: [c_out-chunk mc (128p), n] = sum_cc w[:,cc,mc].T @ xT[cc]
                qt = work.tile([128, 2, N], BF16, tag="qt", name="qt")
                kt = work.tile([128, 2, N], BF16, tag="kt", name="kt")
                for w_r, dst in ((wq_r, qt), (wk_r, kt)):
                    ps_qk = pp.tile([128, 2, N], F32, tag="qk", name="ps_qk")
                    for mc in range(2):
                        for cc in range(2):
                            nc.tensor.matmul(
                                ps_qk[:, mc, :],
                                w_r[:, cc, mc * 128:(mc + 1) * 128],
                                xt[:, cc, :],
                                start=(cc == 0), stop=(cc == 1),
                            )
                    nc.vector.tensor_copy(dst[:], ps_qk[:])
                # v natural: [j-chunk jc (128p), d_all] = sum_cc xT[cc,:,jslice].T @ wv[cc]
                ps_v = vfp.tile([128, 2, N], F32, tag="vf")
                for jc in range(2):
                    for cc in range(2):
                        nc.tensor.matmul(
                            ps_v[:, jc, :],
                            xt[:, cc, jc * 128:(jc + 1) * 128],
                            wv_r[:, cc, :],
                            start=(cc == 0), stop=(cc == 1),
                        )
                nc.scalar.copy(
                    va.rearrange("p jc (h w) -> p jc h w", w=34)[:, :, :, 0:32],
                    ps_v.rearrange("p jc (h d) -> p jc h d", d=32),
                )

                # ---- attention, per 2-head group; attn@v in natural layout ----
                nat = natp.tile([128, 2, H, 64], F32, tag="nat", name="nat")
                for g in range(4):
                    dots = dp.tile([128, 2, 2, N], F32, tag="dots", name="dots")
                    for hh in range(2):
                        h = 2 * g + hh
                        rb = (h % 4) * 32
                        mc = h // 4
                        for jc in range(2):
                            nc.tensor.matmul(
                                dots[:, hh, jc, :],
                                kt[rb:rb + 32, mc, jc * 128:(jc + 1) * 128],
                                qt[rb:rb + 32, mc, :],
                                start=True, stop=True,
                                tile_position=(rb, 0),
                            )
                    eraw = ework.tile([128, 2, 2, N], BF16, tag="eraw",
                                      name="eraw")
                    nc.scalar.activation(eraw[:], dots[:],
                                         mybir.ActivationFunctionType.Exp)
                    e_g = ework.tile([128, 2, 2, N], BF16, tag="e", name="e")
                    mul_eng = nc.gpsimd if g % 2 == 0 else nc.vector
                    mul_eng.tensor_mul(e_g[:], eraw[:],
                                       ebt_sb[:, 2 * g:2 * g + 2, :, :])
                    # out_nat[n, d] (+ sums in column 32) = E^T @ [v | 1]
                    for hh in range(2):
                        h = 2 * g + hh
                        for nck in range(2):
                            for jc in range(2):
                                nc.tensor.matmul(
                                    nat[:, nck, h, 0:34],
                                    e_g[:, hh, jc, nck * 128:(nck + 1) * 128],
                                    va[:, jc, h * 34:(h + 1) * 34],
                                    start=(jc == 0), stop=(jc == 1),
                                )

                return nat

            def batch_tail(b, nat):
                # ---- softmax normalization: per-partition scalars ----
                r_s = work.tile([128, 2, H, 1], F32, tag="r", name="r")
                nc.vector.reciprocal_approx_fast(
                    r_s.rearrange("p nck h o -> p (nck h o)"),
                    nat[:, :, :, 32:33].rearrange("p nck h o -> p (nck h o)"))
                stag = stagp.tile([128, 2, H, 32], F32R, tag="stag", name="stag")
                nc.vector.tensor_mul(
                    stag[:], nat[:, :, :, 0:32],
                    r_s.broadcast_to([128, 2, H, 32]))

                # ---- transpose to [inner, n] for the output projection ----
                sv = stag.rearrange("p nck h d -> p nck (h d)")
                psT = dp.tile([128, 2, 2, 128], F32R, tag="dots", name="psT")
                for nck in range(2):
                    for ic in range(2):
                        nc.tensor.transpose(
                            psT[:, ic, nck, :],
                            sv[:, nck, ic * 128:(ic + 1) * 128],
                            ident_r[:])
                stag_t = stagp.tile([128, 2, 2, 128], F32R, tag="stagT",
                                    name="stagT")
                nc.vector.tensor_copy(stag_t[:], psT[:])

                # ---- output projection ----
                ps_f = vfp.tile([128, 2, N], F32, tag="vf")
                for nck in range(2):
                    for ic in range(2):
                        nc.tensor.matmul(
                            ps_f[:, nck, :],
                            stag_t[:, ic, nck, :],
                            wo_r[:, ic, :],
                            start=(ic == 0), stop=(ic == 1),
                        )
                fout = work.tile([128, 2, N], F32, tag="fout")
                nc.vector.tensor_add(
                    fout[:], ps_f[:],
                    bb[:, None, :].broadcast_to([128, 2, N]),
                )
                nc.sync.dma_start(
                    out=out8[b].rearrange("(ncc p) c -> p ncc c", p=128),
                    in_=fout[:],
                )

            def emit_all():
                xts = [load_xt(b) for b in range(BPC)]
                prev = None
                for b in range(BPC):
                    nat = batch_body(b, xts[b])
                    if prev is not None:
                        batch_tail(b - 1, prev)
                    prev = nat
                batch_tail(BPC - 1, prev)

            if reps == 1:
                emit_all()
            else:
                with tc.For_i(0, reps, 1,
                              hint_engines=(mybir.EngineType.PE,)):
                    emit_all()

    nc.compile()
    return nc


def _prep_shared(w_qkv, bias_table, w_out, b_out, rel_index):
    w_qkv = np.asarray(w_qkv, dtype=np.float32)
    w_q = w_qkv[:, 0:C] * np.float32(SCALE)
    w_k = w_qkv[:, C:2 * C]
    w_v = w_qkv[:, 2 * C:3 * C]
    wq_t = np.ascontiguousarray(w_q.reshape(2, 128, C).transpose(1, 0, 2))
    wk_t = np.ascontiguousarray(w_k.reshape(2, 128, C).transpose(1, 0, 2))
    wv_t = np.ascontiguousarray(w_v.reshape(2, 128, C).transpose(1, 0, 2))

    w_out = np.asarray(w_out, dtype=np.float32)
    wo_t = np.ascontiguousarray(w_out.reshape(2, 128, C).transpose(1, 0, 2))

    # exp of relative-position bias, transposed per head: expBT[h, j, n]
    tbl = np.asarray(bias_table, dtype=np.float32)[np.asarray(rel_index)]
    bias_njh = tbl.reshape(N, N, H)                 # [n, j, h]
    ebt_full = np.exp(bias_njh.transpose(2, 1, 0))  # [h, j, n]
    import ml_dtypes
    ebt_t = np.ascontiguousarray(
        ebt_full.reshape(H, 2, 128, N).transpose(2, 0, 1, 3)).astype(
            ml_dtypes.bfloat16)  # [p, h, jc, n]

    return {
        "wq": wq_t, "wk": wk_t, "wv": wv_t, "wo": wo_t,
        "bo": np.ascontiguousarray(np.asarray(b_out, dtype=np.float32)),
        "ebt": ebt_t,
    }


_NC_CACHE = {}


def get_nc(reps: int = 1):
    if reps not in _NC_CACHE:
        _NC_CACHE[reps] = build_nc(reps)
    return _NC_CACHE[reps]


def make_in_maps(x, w_qkv, bias_table, w_out, b_out, rel_index):
    shared = _prep_shared(w_qkv, bias_table, w_out, b_out, rel_index)
    x = np.asarray(x, dtype=np.float32)
    xs = x.reshape(N_CORES, BPC, N, C).transpose(0, 1, 3, 2)
    xs = np.ascontiguousarray(xs)
    return [{"xt8": xs[i], **shared} for i in range(N_CORES)]


def kernel(x, w_qkv, bias_table, w_out, b_out, rel_index):
    nc = get_nc(1)
    in_maps = make_in_maps(x, w_qkv, bias_table, w_out, b_out, rel_index)
    res = run_bass_kernel_spmd(nc, in_maps, core_ids=list(range(N_CORES)))
    return np.concatenate([res.results[i]["out8"] for i in range(N_CORES)],
                          axis=0)
